# revision 5
# baseline (speedup 1.0000x reference)
"""Trainium2 Bass kernel for nn_MultiHeadAttention_59227599012491.

Reference computation (per batch b):
    xf = x[b].reshape(S, 256)
    q  = softplus(xf @ Wq.T + bq);  k = softplus(xf @ Wk.T + bk)
    v  = xf @ Wv.T + bv
    out = ((q @ k.T) @ v) @ Wo.T + bo          (no softmax!)

No softmax -> attention is associative, and v is linear in x, so v and
the whole G/M weight chain fold away:
    out = q @ M + bo
    HT[c,e] = sum_s x[s,c] k[s,e]          (lhsT = x_nat chunk, rhs = k tile)
    M[e,do] = sum_c HT[c,e] WVO[c,do] + sum_s' Kbar[s',e] u[do]
with WVO = WvT @ WoT and u = Wo bv host-precomputed, and Kbar the
partition-folded k sum; the bv rank-1 correction rides the M matmul as
128-contract channels against a host-replicated u tile.

Sharding (v4): 8 cores = 4 batches x 2 sequence-halves, and the k/H
DUPLICATION of earlier versions is gone: each core computes k, softplus,
H and Kbar only for ITS 2048 rows, then the pair exchanges partial
H+Kbar (one [128,768] fp16 SBUF->SBUF remote DMA, ~192KB) over the
intra-chip SDMA fabric while the ACT engine is busy with q softplus --
the exchange latency hides completely. M sums own+peer partials as
extra matmul channels. This halves the pacing ACT engine's softplus
work (the whole-k duplication was the floor of v1-v3).

Remote exchange mechanics: relative-dest remote_dma_broadcast
(Delta-rid=0, Delta-tpb=1 -> pair partner; verified on hardware), the
payload split over 4 slot positions (4 SDMA engine pairs), descriptors
prepared on the GPSIMD SWDGE ring and fired with one trigger. The
cross-core semaphore waits CANNOT live inside a TileContext (the tile
scheduler's single-core sim would deadlock), so the program is three
regions: TC1 [DMAs, k-proj, softplus, H, Kbar, q] -> raw [engine-order
sem_incs, descriptor prep+trigger, remote-sem waits] -> TC2 [M, out].
Engine queues are in-order, so a raw wait gates everything TC2 puts on
that engine. Exchange buffers and every tensor TC2 touches live in RAW
SBUF tensors (concrete addresses) -- TC-pool tile APs are symbolic and
cannot be referenced by raw instructions.

Other carried-over scheduling facts (all measured): 6 dependency-free
priming matmuls ramp the PE clock out of its low pstate during the DMA
head; the first k tile's x columns load as a dedicated small chunk
(first matmul at ~8us instead of ~12); biasc is ordered BEFORE wk in
the sync-ring FIFO (the DVE's first bias-add gates the whole pipe, and
biasc was measured landing ~2.7us late when queued after the x/wk
transfers); PSUM accumulation groups never share a bank; activation
tables steered so the one table holding Exp AND Ln loads exactly once;
Kbar partial sums ride the DVE as per-batch halving trees (GPSIMD
tensor ops measured ~730ns per [128,256] add - too slow).
"""

import numpy as np

S = 4096
SQ = 2048  # sequence rows per core (own half)
D = 256
P = 128
IT = D // P  # 2 input-dim tiles
DT = D // P  # 2 d-model tiles
NS = SQ // P  # 16 sequence tiles per core
BLK = 512  # free-dim block for qT
N_CORES = 8
XFW = 2 * D + D  # xfer width: H (2 chunks) + Kbar

MM_DTYPE_NAME = "float16"

_CACHE = {}


def _patched_act_tables(orig_fn):
    def patched(arch):
        tabs = orig_fn(arch)
        return {
            name: (s if name == "natural_log_exp_and_others" else set())
            for name, s in tabs.items()
        }

    return patched


def _build_nc():
    import concourse.bacc as bacc
    import concourse.mybir as mybir
    import concourse.tile as tile

    FP = mybir.dt.float32
    FR = getattr(mybir.dt, MM_DTYPE_NAME)
    AF = mybir.ActivationFunctionType
    ADD = mybir.AluOpType.add

    nc = bacc.Bacc("TRN2", target_bir_lowering=False, debug=False, num_devices=1)

    xbT_d = nc.declare_dram_parameter("xbT", [D, SQ], FR, isOutput=False)
    xnat_d = nc.declare_dram_parameter("xnat", [SQ, D], FR, isOutput=False)
    wkp_d = nc.declare_dram_parameter("wkp", [P, IT * D], FR, isOutput=False)
    # wq it-blocks | WVO c-blocks: [128, 1024]
    wqop_d = nc.declare_dram_parameter("wqop", [P, 1024], FR, isOutput=False)
    ut_d = nc.declare_dram_parameter("ut", [P, D], FR, isOutput=False)
    bias_d = nc.declare_dram_parameter("biasc", [P, 4 + D], FP, isOutput=False)
    outp_d = nc.declare_dram_parameter("outp", [P, 2 * SQ], FR, isOutput=True)

    # raw SBUF tensors: referenced by raw instructions and/or TC2
    wqo_sb = nc.alloc_sbuf_tensor("wqo_sb", [P, 1024], FR).ap()
    ut_sb = nc.alloc_sbuf_tensor("ut_sb", [P, D], FR).ap()
    biasc = nc.alloc_sbuf_tensor("biasc_sb", [P, 4 + D], FP).ap()
    qT_sb = nc.alloc_sbuf_tensor("qT_sb", [P, DT, SQ], FR).ap()
    outT_sb = nc.alloc_sbuf_tensor("outT_sb", [P, DT, SQ], FR).ap()
    xfer = nc.alloc_sbuf_tensor("xfer_sb", [P, XFW], FR).ap()
    recv = nc.alloc_sbuf_tensor("recv_sb", [P, XFW], FR).ap()
    M_sb = nc.alloc_sbuf_tensor("M_sb", [P, DT, D], FR).ap()
    bias_sb = biasc[:, 0:4]
    bk_bc = biasc[:, 4 : 4 + D]

    remote_sem = nc.alloc_semaphore("rsem")
    local_sem = nc.alloc_semaphore("lsem")
    vready = nc.alloc_semaphore("vready")
    qdone = nc.alloc_semaphore("qdone")
    prep_sem = nc.alloc_semaphore("prep")
    trig_sem = nc.alloc_semaphore("trig")

    def mm(psum, lhsT, rhs, start, stop):
        nc.tensor.matmul(psum, lhsT, rhs, start=start, stop=stop)

    # ---------------- TC1: k-proj, softplus, H, Kbar, q ----------------
    with tile.TileContext(nc) as tc:
        with (
            tc.tile_pool(name="w", bufs=1) as wpool,
            tc.tile_pool(name="big", bufs=1) as big,
            tc.tile_pool(name="tmp", bufs=4) as tpool,
            tc.tile_pool(name="psQ", bufs=2, space="PSUM") as psQ,
            tc.tile_pool(name="psK", bufs=3, space="PSUM") as psK,
            tc.tile_pool(name="psH", bufs=1, space="PSUM") as psH,
            tc.tile_pool(name="psP", bufs=1, space="PSUM") as psP,
        ):
            wk_sb = wpool.tile([P, IT * D], FR, tag="wk")
            xbT_sb = big.tile([P, IT, SQ], FR, tag="xbT")
            xnat_sb = big.tile([P, NS, D], FR, tag="xnat")
            prime_sb = wpool.tile([P, 512], FR, tag="prime")

            # PE pstate priming during the DMA head
            nc.gpsimd.memset(prime_sb[:, :], 0.0)
            psprime = psP.tile([P, 512], FP, tag="psP")
            for i in range(6):
                mm(psprime[:, :], prime_sb[:, 0:P], prime_sb[:, :], True, True)

            # --- input DMAs, sync-ring FIFO order is priority ---
            for it in range(IT):
                nc.sync.dma_start(
                    xbT_sb[:, it, 0:256], xbT_d.ap()[it * P : (it + 1) * P, 0:256]
                )
            nc.sync.dma_start(biasc[:, :], bias_d.ap()[:, :])
            nc.sync.dma_start(wk_sb[:, :], wkp_d.ap()[:, :])
            for it in range(IT):
                nc.sync.dma_start(
                    xbT_sb[:, it, 256:1024], xbT_d.ap()[it * P : (it + 1) * P, 256:1024]
                )
            nc.sync.dma_start(wqo_sb[:, :], wqop_d.ap()[:, :])
            for it in range(IT):
                nc.sync.dma_start(
                    xbT_sb[:, it, 1024:2048],
                    xbT_d.ap()[it * P : (it + 1) * P, 1024:2048],
                )
            xn_ap = xnat_d.ap().rearrange("(t p) c -> p t c", p=P)
            nc.sync.dma_start(xnat_sb[:, 0:8, :], xn_ap[:, 0:8, :])
            nc.sync.dma_start(xnat_sb[:, 8:16, :], xn_ap[:, 8:16, :])
            nc.sync.dma_start(ut_sb[:, :], ut_d.ap()[:, :])

            k_sb = big.tile([P, NS, D], FR, tag="k")

            # persistent H accumulators: one full PSUM bank per c-chunk
            psH0 = psH.tile([P, 512], FP, tag="psH0")
            psH1 = psH.tile([P, 512], FP, tag="psH1")
            psHc = [psH0, psH1]

            xferK = xfer[:, 2 * D : XFW]  # Kbar slot of the exchange buffer

            def q_block(dt, half):
                tmp = tpool.tile([P, 2, BLK], FP, tag="tmpq")
                for c in range(2):
                    blk = 2 * half + c
                    ss = slice(blk * BLK, (blk + 1) * BLK)
                    ps = psQ.tile([P, BLK], FP, tag="psQ")
                    for it in range(IT):
                        mm(
                            ps[:, :],
                            wqo_sb[:, it * D + dt * P : it * D + (dt + 1) * P],
                            xbT_sb[:, it, ss],
                            it == 0,
                            it == IT - 1,
                        )
                    nc.scalar.activation(
                        tmp[:, c, :], ps[:, :], AF.Exp, bias=bias_sb[:, dt : dt + 1]
                    )
                nc.scalar.activation(
                    qT_sb[:, dt, 2 * half * BLK : 2 * (half + 1) * BLK],
                    tmp[:, :, :].rearrange("p a b -> p (a b)"),
                    AF.Ln,
                    bias=1.0,
                )

            # batch ramp over 16 tiles; q blocks interleave between batches
            SPB = {1: 2, 3: 2, 7: 4, 11: 4, 13: 2, 15: 2}
            QAT = {2: (0, 0), 5: (0, 1), 9: (1, 0), 13: (1, 1)}

            first_kb = True
            for t in range(NS):
                ts = slice(t * P, (t + 1) * P)
                ps = psK.tile([P, 512], FP, tag="psK")
                for it in range(IT):
                    mm(
                        ps[:, 0:D],
                        xbT_sb[:, it, ts],
                        wk_sb[:, it * D : (it + 1) * D],
                        it == 0,
                        it == IT - 1,
                    )
                nc.vector.tensor_tensor(k_sb[:, t, :], ps[:, 0:D], bk_bc, op=ADD)
                bsz = SPB.get(t, 0)
                if bsz:
                    tt = slice(t - bsz + 1, t + 1)
                    tmp = tpool.tile([P, bsz, D], FP, tag=f"tmpk{bsz}")
                    nc.scalar.activation(tmp[:, :, :], k_sb[:, tt, :], AF.Exp)
                    nc.scalar.activation(k_sb[:, tt, :], tmp[:, :, :], AF.Ln, bias=1.0)
                    for t2 in range(t - bsz + 1, t + 1):
                        for c in range(IT):
                            mm(
                                psHc[c][:, 0:D],
                                xnat_sb[:, t2, c * P : (c + 1) * P],
                                k_sb[:, t2, :],
                                t2 == 0,
                                t2 == NS - 1,
                            )
                    # Kbar partial into the exchange buffer (DVE halving tree)
                    a = t - bsz + 1
                    if bsz == 2:
                        if first_kb:
                            nc.vector.tensor_tensor(
                                xferK, k_sb[:, a, :], k_sb[:, a + 1, :], op=ADD
                            )
                        else:
                            t2b = tpool.tile([P, D], FR, tag="kb2")
                            nc.vector.tensor_tensor(
                                t2b[:, :], k_sb[:, a, :], k_sb[:, a + 1, :], op=ADD
                            )
                            nc.vector.tensor_tensor(xferK, xferK, t2b[:, :], op=ADD)
                    else:  # bsz == 4
                        t4 = tpool.tile([P, 2, D], FR, tag="kb4")
                        nc.vector.tensor_tensor(
                            t4[:, :, :],
                            k_sb[:, a : a + 2, :],
                            k_sb[:, a + 2 : a + 4, :],
                            op=ADD,
                        )
                        t2b = tpool.tile([P, D], FR, tag="kb2")
                        nc.vector.tensor_tensor(
                            t2b[:, :], t4[:, 0, :], t4[:, 1, :], op=ADD
                        )
                        nc.vector.tensor_tensor(xferK, xferK, t2b[:, :], op=ADD)
                    first_kb = False
                if t in QAT:
                    q_block(*QAT[t])

            # evict H partials into the exchange buffer
            for c in range(IT):
                nc.vector.tensor_copy(xfer[:, c * D : (c + 1) * D], psHc[c][:, 0:D])

    # ---------------- raw region: pair exchange ----------------
    # engine queues are in-order: these incs fire after ALL TC1 work on
    # that engine; the waits gate everything TC2 enqueues afterwards
    nc.vector.sem_inc(vready, 1)  # H casts + Kbar in xfer
    nc.scalar.sem_inc(qdone, 1)  # all softplus (k and q) done
    nc.gpsimd.wait_ge(vready, 1)
    XC = XFW // 4
    for s in range(4):
        rdests = [None] * 8
        rdests[s] = (0, 1)
        nc.gpsimd.remote_dma_broadcast(
            recv[:, s * XC : (s + 1) * XC],
            xfer[:, s * XC : (s + 1) * XC],
            remote_sem,
            local_sem,
            rdests=rdests,
        ).then_inc(prep_sem, 1)
    nc.gpsimd.wait_ge(prep_sem, 4)
    nc.gpsimd.trigger_dma(count=4).then_inc(trig_sem, 1)
    nc.tensor.wait_ge(remote_sem, 8)  # peer's partials arrived
    nc.tensor.wait_ge(vready, 1)  # own xfer readable
    nc.tensor.wait_ge(qdone, 1)  # qT complete (out phase reads it)

    # ---------------- TC2: M and the out phase ----------------
    with tile.TileContext(nc) as tc2:
        with (
            tc2.tile_pool(name="psM", bufs=2, space="PSUM") as psM,
            tc2.tile_pool(name="psO", bufs=3, space="PSUM") as psO,
        ):
            # M[e,do] = sum_c (HT_own+HT_peer)[c,e] WVO[c,do]
            #         + sum_s' (Kbar_own+Kbar_peer)[s',e] u[do]
            for et in range(DT):
                es = slice(et * P, (et + 1) * P)
                ps = psM.tile([P, 512], FP, tag="psM")
                for c in range(IT):
                    mm(
                        ps[:, 0:D],
                        xfer[:, c * D : c * D + D][:, es],
                        wqo_sb[:, 512 + c * D : 512 + (c + 1) * D],
                        c == 0,
                        False,
                    )
                    mm(
                        ps[:, 0:D],
                        recv[:, c * D : c * D + D][:, es],
                        wqo_sb[:, 512 + c * D : 512 + (c + 1) * D],
                        False,
                        False,
                    )
                mm(ps[:, 0:D], xfer[:, 2 * D : XFW][:, es], ut_sb[:, :], False, False)
                mm(ps[:, 0:D], recv[:, 2 * D : XFW][:, es], ut_sb[:, :], False, True)
                nc.vector.tensor_copy(M_sb[:, et, :], ps[:, 0:D])

            # outT[do, s] = M^T q^T + bo
            for dot in range(DT):
                for blk in range(SQ // BLK):
                    ss = slice(blk * BLK, (blk + 1) * BLK)
                    ps = psO.tile([P, BLK], FP, tag="psO")
                    for et in range(DT):
                        mm(
                            ps[:, :],
                            M_sb[:, et, dot * P : (dot + 1) * P],
                            qT_sb[:, et, ss],
                            et == 0,
                            et == DT - 1,
                        )
                    if dot == DT - 1 and blk == SQ // BLK - 1:
                        nc.scalar.activation(
                            outT_sb[:, dot, blk * BLK : blk * BLK + 256],
                            ps[:, 0:256],
                            AF.Identity,
                            bias=bias_sb[:, 2 + dot : 3 + dot],
                        )
                        nc.vector.tensor_scalar_add(
                            outT_sb[:, dot, blk * BLK + 256 : (blk + 1) * BLK],
                            ps[:, 256:512],
                            bias_sb[:, 2 + dot : 3 + dot],
                        )
                    else:
                        nc.vector.tensor_scalar_add(
                            outT_sb[:, dot, ss], ps[:, :], bias_sb[:, 2 + dot : 3 + dot]
                        )
                    if dot == DT - 1 and blk >= SQ // BLK - 2:
                        off = dot * SQ + blk * BLK
                        src_ap = outT_sb[:, dot, blk * BLK : (blk + 1) * BLK]
                        if blk == SQ // BLK - 1:
                            nc.sync.dma_start(
                                outp_d.ap()[0:64, off : off + BLK], src_ap[0:64, :]
                            )
                            nc.scalar.dma_start(
                                outp_d.ap()[64:P, off : off + BLK], src_ap[64:P, :]
                            )
                        else:
                            nc.sync.dma_start(outp_d.ap()[:, off : off + BLK], src_ap)
                    elif blk % 2 == 1:
                        off = dot * SQ + (blk - 1) * BLK
                        src_ap = outT_sb[:, dot, (blk - 1) * BLK : (blk + 1) * BLK]
                        nc.sync.dma_start(outp_d.ap()[:, off : off + 2 * BLK], src_ap)

    import concourse.hw_specs as hw_specs

    orig = bacc.get_activation_tables
    bacc.get_activation_tables = _patched_act_tables(hw_specs.get_activation_tables)
    try:
        nc.compile()
    finally:
        bacc.get_activation_tables = orig
    return nc


def _get_nc():
    nc = _CACHE.get("nc")
    if nc is None:
        nc = _build_nc()
        _CACHE["nc"] = nc
    return nc


def make_in_maps(x, Wq, bq, Wk, bk, Wv, bv, Wo, bo):
    B = x.shape[0]
    mmnp = np.float16
    xf = np.asarray(x, dtype=np.float32).reshape(B, S, D)
    xfT = np.ascontiguousarray(xf.transpose(0, 2, 1).astype(mmnp))
    xnat = np.ascontiguousarray(xf.astype(mmnp))
    wk2 = np.asarray(Wk, mmnp).T
    wkp = np.ascontiguousarray(np.hstack([wk2[0:P], wk2[P:D]]))  # [128, 512]
    wq2 = np.asarray(Wq, mmnp).T
    wvo = (np.asarray(Wv, np.float64).T @ np.asarray(Wo, np.float64).T).astype(mmnp)
    wqop = np.ascontiguousarray(
        np.hstack([wq2[0:P], wq2[P:D], wvo[0:P], wvo[P:D]])
    )  # [128, (it0 wq|it1 wq|c0 WVO|c1 WVO)]
    u = (np.asarray(Wo, np.float64) @ np.asarray(bv, np.float64)).astype(mmnp)
    ut = np.ascontiguousarray(np.tile(u, (P, 1)))
    biasc = np.ascontiguousarray(
        np.hstack(
            [
                np.stack(
                    [
                        np.asarray(bq, np.float32)[0:P],
                        np.asarray(bq, np.float32)[P:D],
                        np.asarray(bo, np.float32)[0:P],
                        np.asarray(bo, np.float32)[P:D],
                    ],
                    axis=1,
                ),
                np.tile(np.asarray(bk, np.float32), (P, 1)),
            ]
        )
    )
    shared = {
        "wkp": wkp,
        "wqop": wqop,
        "ut": ut,
        "biasc": biasc,
    }
    in_maps = []
    for c in range(N_CORES):
        b, h = divmod(c, 2)
        sl = slice(h * SQ, (h + 1) * SQ)
        in_maps.append(
            {
                "xbT": np.ascontiguousarray(xfT[b][:, sl]),
                "xnat": np.ascontiguousarray(xnat[b][sl]),
                **shared,
            }
        )
    return in_maps


def assemble_out(results, x_shape):
    B, S_, H, W = x_shape
    out = np.empty((B, S_, D), np.float32)
    for c in range(N_CORES):
        b, h = divmod(c, 2)
        outp = results[c]["outp"]  # [128, 2*SQ] fp16: [p, dot*SQ + s]
        v = outp.reshape(P, DT, SQ).astype(np.float32)
        out[b, h * SQ : (h + 1) * SQ] = v.transpose(2, 1, 0).reshape(SQ, D)
    return out.reshape(B, S_, H, W)


def kernel(x, Wq, bq, Wk, bk, Wv, bv, Wo, bo, _trace=False):
    from concourse.bass_utils import run_bass_kernel_spmd

    nc = _get_nc()
    in_maps = make_in_maps(x, Wq, bq, Wk, bk, Wv, bv, Wo, bo)
    res = run_bass_kernel_spmd(nc, in_maps, list(range(N_CORES)), trace=_trace)
    out = assemble_out(res.results, x.shape)
    if _trace:
        _CACHE["last_result"] = res
    return out


# revision 7
# speedup vs baseline: 1.3500x; 1.3500x over previous
"""Trainium2 Bass kernel for nn_MultiHeadAttention_59227599012491.

Reference computation (per batch b):
    xf = x[b].reshape(S, 256)
    q  = softplus(xf @ Wq.T + bq);  k = softplus(xf @ Wk.T + bk)
    v  = xf @ Wv.T + bv
    weight = q @ k.T            (no softmax!)
    result = weight @ v
    out    = result @ Wo.T + bo

Because there is no softmax, attention is associative:
    result = (q @ k.T) @ v = q @ (k.T @ v) = q @ G,   G: [256, 256]
    out    = q @ (G @ Wo.T) + bo = q @ M + bo
so the S x S score matrix never needs to be materialized. Per-core work
drops to a handful of [*, 256] x [256, 256] matmuls; the kernel is
jointly engine-bound (PE ~32 us, ACT ~29 us, DVE ~28 us busy/core).

Hard-won scheduling facts (all measured): the three DMA-issuing rings
(sync/scalar HWDGE, gpsimd SWDGE) share the same 16 physical DMA
engines, so the single sync-ring FIFO *is* the priority mechanism --
spreading input loads across rings only slows the critical pieces.
GPSIMD cannot touch PSUM; two PSUM accumulation groups must not share
a bank; an NRT AllReduce has ~17 us fixed rendezvous latency. fp32
PSUM evictions are uop-pinned to 1x on the DVE (one PSUM read port)
and ScalarE is 1 elem/cycle for all dtypes, so the elementwise floors
are hard. DMA-order rule: FIFO position in the single sync ring is
priority; order each engine's gate tensor so all gates of the first
dependent op complete SIMULTANEOUSLY (biasc right after x0 balances
the DVE's bias gate against its matmul gate -- either extreme costs
1-3 us on the DVE-paced loop). Descriptor count = DRAM row count per
tensor; the head floor is 512 descriptors (~5.5 us) before the first
matmul.

Sharding: B=4 batches x 2 query-halves -> 8 cores, no collectives.
(An NRT AllReduce of the tiny M matrix was measured at ~17 us fixed
rendezvous latency on this runtime -- more than the k/v/G dedup saves --
so each core recomputes k/v/G/M for its whole batch and only the
query/output rows are split across the pair.)

Layouts (PE computes out = lhsT.T @ rhs, contracting partition dim):
    xbT  [256, 4096]  x[b] transposed on host (queries first SQ cols)
    qT   [256, 2048]  lhsT = WqT tile, rhs = xbT     (softplus via ACT,
                      bias per-partition, fused into the Exp pass)
    kv   [4096, 512]  k and v fused: rhs = [WkT | WvT], one stationary
                      xbT tile per row tile serves both. +[bk|bv] via a
                      single DVE add; softplus on the k half in-place
                      (ACT Exp then Ln(1+t); batch-size ramp 2,4,8,8
                      then 4,2,2,2 -- small first so the ~98%-saturated
                      ACT engine starts ASAP, small last so the final
                      softplus -> GT -> M -> out chain stays short)
    GT   [256, 256]   GT[d,e] = sum_s v[s,d] k[s,e]: lhsT = v t, rhs = k t
    M    [256, 256]   M[e,do] = sum_d GT[d,e] WoT[d,do]: lhsT = GT, rhs = WoT
    outT [256, 2048]  transposed output: lhsT = M block (stationary,
                      reused across s), rhs = qT chunk; bo is then
                      per-partition (DVE tensor_scalar_add) and the fp16
                      dump has 2 KB descriptor runs -- half the output
                      bytes and a quarter of the descriptors vs the
                      natural-layout fp32 store; host un-transposes

The tile scheduler interleaves the qT/GT/out matmuls into the DVE-paced
kv loop's PE gaps, so the PE runs at ~91% occupancy over its window;
the engines are jointly near-saturated (PE ~32 us, ACT ~29 us, DVE
~28 us busy per core) and the phase structure below measures faster
than every explicitly-interleaved variant tried.

The activation-table pass is steered to `natural_log_exp_and_others`
(the only set holding Exp AND Ln) so the ACT engine loads its PWP table
once instead of reloading per activation (24 loads ~= 30us saved).
"""

import numpy as np

S = 4096
SQ = 2048  # query rows per core
D = 256
P = 128
IT = D // P  # 2 input-dim tiles
DT = D // P  # 2 d-model tiles
NS = S // P  # 32 sequence tiles
BLK = 512  # free-dim block for qT
N_CORES = 8

MM_DTYPE_NAME = "float16"

_CACHE = {}


def _patched_act_tables(orig_fn):
    def patched(arch):
        tabs = orig_fn(arch)
        return {
            name: (s if name == "natural_log_exp_and_others" else set())
            for name, s in tabs.items()
        }

    return patched


def _build_nc():
    import concourse.bacc as bacc
    import concourse.mybir as mybir
    import concourse.tile as tile

    FP = mybir.dt.float32
    FR = getattr(mybir.dt, MM_DTYPE_NAME)
    AF = mybir.ActivationFunctionType
    ADD = mybir.AluOpType.add

    nc = bacc.Bacc("TRN2", target_bir_lowering=False, debug=False, num_devices=1)

    xbT_d = nc.declare_dram_parameter("xbT", [D, S], FR, isOutput=False)
    # weights host-packed with it-blocks side by side: 2 KB descriptor rows
    wkvp_d = nc.declare_dram_parameter("wkvp", [P, 1024], FR, isOutput=False)
    wqop_d = nc.declare_dram_parameter("wqop", [P, 1024], FR, isOutput=False)
    # all biases in one [128, 516] fp32 tensor (cols 0:4 = bqT|boT,
    # 4:516 = host-replicated [bk|bv] row): one early DMA, 2 KB rows,
    # so the DVE and ACT queue gates both clear right after wkv
    bias_d = nc.declare_dram_parameter("biasc", [P, 4 + 2 * D], FP, isOutput=False)
    outp_d = nc.declare_dram_parameter("outp", [P, 2 * SQ], FR, isOutput=True)

    def mm(psum, lhsT, rhs, start, stop):
        nc.tensor.matmul(psum, lhsT, rhs, start=start, stop=stop)

    with tile.TileContext(nc) as tc:
        with (
            tc.tile_pool(name="w", bufs=1) as wpool,
            tc.tile_pool(name="big", bufs=1) as big,
            tc.tile_pool(name="tmp", bufs=4) as tpool,
            tc.tile_pool(name="psQ", bufs=3, space="PSUM") as psQ,
            tc.tile_pool(name="psKV", bufs=3, space="PSUM") as psKV,
            tc.tile_pool(name="psG", bufs=2, space="PSUM") as psG,
        ):
            # PE pstate priming: dependency-free matmuls on a scratch tile
            # ramp the clock out of its low pstate during the DMA head
            prime_sb = wpool.tile([P, 512], FR, tag="prime")
            nc.vector.memset(prime_sb[:, :], 0.0)
            psprime = psG.tile([P, D], FP, tag="psG")
            for _pi in range(6):
                nc.tensor.matmul(psprime[:, :], prime_sb[:, 0:P], prime_sb[:, 0:D], start=True, stop=True)
            wkv_sb = wpool.tile([P, 2 * 512], FR, tag="wkv")
            wqo_sb = wpool.tile([P, 2 * 512], FR, tag="wqo")
            xbT_sb = big.tile([P, IT, S], FR, tag="xbT")
            biasc = wpool.tile([P, 4 + 2 * D], FP, tag="biasc")
            bias_sb = biasc[:, 0:4]
            bc_bkv = biasc[:, 4 : 4 + 2 * D]
            b_bc = {"bkv": bc_bkv}
            # first k tile's x columns as a dedicated small chunk, then
            # biasc BEFORE wkv: the DVE's first bias-add gates the pipe and
            # biasc was measured landing ~2.7us late when queued later
            for it in range(IT):
                nc.sync.dma_start(
                    xbT_sb[:, it, 0:256], xbT_d.ap()[it * P : (it + 1) * P, 0:256]
                )
            nc.sync.dma_start(biasc[:, :], bias_d.ap()[:, :])
            nc.sync.dma_start(wkv_sb[:, :], wkvp_d.ap()[:, :])
            for it in range(IT):
                nc.sync.dma_start(
                    xbT_sb[:, it, 256:1024], xbT_d.ap()[it * P : (it + 1) * P, 256:1024]
                )
            nc.sync.dma_start(wqo_sb[:, :], wqop_d.ap()[:, :])
            for it in range(IT):
                nc.sync.dma_start(
                    xbT_sb[:, it, 1024:2048], xbT_d.ap()[it * P : (it + 1) * P, 1024:2048]
                )
            for it in range(IT):
                nc.sync.dma_start(
                    xbT_sb[:, it, 2048:3072], xbT_d.ap()[it * P : (it + 1) * P, 2048:3072]
                )
            for it in range(IT):
                nc.sync.dma_start(
                    xbT_sb[:, it, 3072:4096], xbT_d.ap()[it * P : (it + 1) * P, 3072:4096]
                )

            kv_sb = big.tile([P, 2, NS, D], FR, tag="kv")
            qT_sb = big.tile([P, DT, SQ], FR, tag="qT")
            outT_sb = big.tile([P, DT, SQ], FR, tag="outT")
            GT_sb = wpool.tile([P, DT, D], FR, tag="GT")
            M_sb = wpool.tile([P, DT, D], FR, tag="M")

            for t in range(NS):
                ts = slice(t * P, (t + 1) * P)
                ps = psKV.tile([P, 2 * D], FP, tag="psKV")
                for it in range(IT):
                    mm(ps[:, :], xbT_sb[:, it, ts], wkv_sb[:, it * 512 : (it + 1) * 512], it == 0, it == IT - 1)
                nc.vector.tensor_tensor(
                    kv_sb[:, :, t, :], ps[:, :].rearrange("p (j d) -> p j d", j=2),
                    b_bc["bkv"][:, :].rearrange("p (j d) -> p j d", j=2), op=ADD,
                )
                # batch-size ramp: 2,4,8,8,8 then 2 at the end -- early
                # batches small so the saturated ACT engine starts ASAP,
                # final batches small so the softplus->GT->M->out chain
                # after the last kv tile stays short
                SPB = {1: 2, 5: 4, 13: 8, 21: 8, 25: 4, 27: 2, 29: 2, 31: 2}
                bsz = SPB.get(t, 0)
                if bsz:
                    tt = slice(t - bsz + 1, t + 1)
                    tmp = tpool.tile([P, bsz, D], FP, tag=f"tmpk{bsz}")
                    nc.scalar.activation(tmp[:, :, :], kv_sb[:, 0, tt, :], AF.Exp)
                    nc.scalar.activation(kv_sb[:, 0, tt, :], tmp[:, :, :], AF.Ln, bias=1.0)

            for dt in range(DT):
                for half in range(SQ // (2 * BLK)):
                    tmp = tpool.tile([P, 2, BLK], FP, tag="tmpq")
                    for c in range(2):
                        blk = 2 * half + c
                        ss = slice(blk * BLK, (blk + 1) * BLK)
                        ps = psQ.tile([P, BLK], FP, tag="psQ")
                        for it in range(IT):
                            mm(ps[:, :], wqo_sb[:, it * D + dt * P : it * D + (dt + 1) * P], xbT_sb[:, it, ss], it == 0, it == IT - 1)
                        nc.scalar.activation(
                            tmp[:, c, :], ps[:, :], AF.Exp, bias=bias_sb[:, dt : dt + 1]
                        )
                    nc.scalar.activation(
                        qT_sb[:, dt, 2 * half * BLK : 2 * (half + 1) * BLK],
                        tmp[:, :, :].rearrange("p a b -> p (a b)"),
                        AF.Ln,
                        bias=1.0,
                    )

            for dt in range(DT):
                vs = slice(dt * P, (dt + 1) * P)
                ps = psG.tile([P, D], FP, tag="psG")
                for t in range(NS):
                    mm(ps[:, :], kv_sb[:, 1, t, vs], kv_sb[:, 0, t, :], t == 0, t == NS - 1)
                nc.vector.tensor_copy(GT_sb[:, dt, :], ps[:, :])

            for et in range(DT):
                es = slice(et * P, (et + 1) * P)
                ps = psG.tile([P, D], FP, tag="psG")
                for dt in range(DT):
                    mm(ps[:, :], GT_sb[:, dt, es], wqo_sb[:, 512 + dt * D : 512 + (dt + 1) * D], dt == 0, dt == DT - 1)
                nc.vector.tensor_copy(M_sb[:, et, :], ps[:, :])

            # outT[do, s] = M^T q^T + bo: lhsT = M block (stationary,
            # reused across s), per-partition bo bias on the DVE, fp16
            # transposed dump with 2 KB descriptor runs
            for dot in range(DT):
                for blk in range(SQ // BLK):
                    ss = slice(blk * BLK, (blk + 1) * BLK)
                    ps = psQ.tile([P, BLK], FP, tag="psQ")
                    for et in range(DT):
                        mm(
                            ps[:, :],
                            M_sb[:, et, dot * P : (dot + 1) * P],
                            qT_sb[:, et, ss],
                            et == 0,
                            et == DT - 1,
                        )
                    if dot == DT - 1 and blk == SQ // BLK - 1:
                        # very last chunk: split the eviction across the idle
                        # ACT and DVE so the serial tail halves
                        nc.scalar.activation(
                            outT_sb[:, dot, blk * BLK : blk * BLK + 256],
                            ps[:, 0:256],
                            AF.Identity,
                            bias=bias_sb[:, 2 + dot : 3 + dot],
                        )
                        nc.vector.tensor_scalar_add(
                            outT_sb[:, dot, blk * BLK + 256 : (blk + 1) * BLK],
                            ps[:, 256:512],
                            bias_sb[:, 2 + dot : 3 + dot],
                        )
                    elif dot == DT - 1:
                        # ACT is drained by now; Identity(in + bo) keeps the
                        # final eviction off the backlogged DVE queue
                        nc.scalar.activation(
                            outT_sb[:, dot, ss],
                            ps[:, :],
                            AF.Identity,
                            bias=bias_sb[:, 2 + dot : 3 + dot],
                        )
                    else:
                        nc.vector.tensor_scalar_add(
                            outT_sb[:, dot, ss], ps[:, :], bias_sb[:, 2 + dot : 3 + dot]
                        )
                    if dot == DT - 1 and blk >= SQ // BLK - 2:
                        # ship the last two chunks individually so only one
                        # chunk's descriptors remain after the final eviction
                        off = dot * SQ + blk * BLK
                        src_ap = outT_sb[:, dot, blk * BLK : (blk + 1) * BLK]
                        if blk == SQ // BLK - 1:
                            nc.sync.dma_start(
                                outp_d.ap()[0:64, off : off + BLK], src_ap[0:64, :]
                            )
                            nc.scalar.dma_start(
                                outp_d.ap()[64:P, off : off + BLK], src_ap[64:P, :]
                            )
                        else:
                            nc.sync.dma_start(outp_d.ap()[:, off : off + BLK], src_ap)
                    elif blk % 2 == 1:
                        off = dot * SQ + (blk - 1) * BLK
                        src_ap = outT_sb[:, dot, (blk - 1) * BLK : (blk + 1) * BLK]
                        nc.sync.dma_start(
                            outp_d.ap()[:, off : off + 2 * BLK], src_ap
                        )

    import concourse.hw_specs as hw_specs

    orig = bacc.get_activation_tables
    bacc.get_activation_tables = _patched_act_tables(hw_specs.get_activation_tables)
    try:
        nc.compile()
    finally:
        bacc.get_activation_tables = orig
    return nc


def _get_nc():
    nc = _CACHE.get("nc")
    if nc is None:
        nc = _build_nc()
        _CACHE["nc"] = nc
    return nc


def make_in_maps(x, Wq, bq, Wk, bk, Wv, bv, Wo, bo):
    B = x.shape[0]
    mmnp = np.float16
    xf = np.asarray(x, dtype=np.float32).reshape(B, S, D)
    xfT = np.ascontiguousarray(xf.transpose(0, 2, 1).astype(mmnp))
    wkv2 = np.hstack([np.asarray(Wk, mmnp).T, np.asarray(Wv, mmnp).T])  # [256, 512]
    wkvp = np.ascontiguousarray(
        wkv2.reshape(2, P, 512).transpose(1, 0, 2).reshape(P, 1024)
    )
    wq2 = np.asarray(Wq, mmnp).T
    wo2 = np.asarray(Wo, mmnp).T
    wqop = np.ascontiguousarray(
        np.hstack([wq2[0:P], wq2[P:D], wo2[0:P], wo2[P:D]])
    )  # [128, (it0 wq|it1 wq|dt0 wo|dt1 wo)]
    bkvrow = np.concatenate([np.asarray(bk, np.float32), np.asarray(bv, np.float32)])
    biasc = np.ascontiguousarray(
        np.hstack(
            [
                np.stack(
                    [
                        np.asarray(bq, np.float32)[0:P],
                        np.asarray(bq, np.float32)[P:D],
                        np.asarray(bo, np.float32)[0:P],
                        np.asarray(bo, np.float32)[P:D],
                    ],
                    axis=1,
                ),
                np.tile(bkvrow, (P, 1)),
            ]
        )
    )
    shared = {
        "wkvp": wkvp,
        "wqop": wqop,
        "biasc": biasc,
    }
    in_maps = []
    for c in range(N_CORES):
        b, h = divmod(c, 2)
        xT = xfT[b]
        if h == 1:
            xT = np.concatenate([xT[:, SQ:], xT[:, :SQ]], axis=1)
        in_maps.append({"xbT": np.ascontiguousarray(xT), **shared})
    return in_maps


def assemble_out(results, x_shape):
    B, S_, H, W = x_shape
    out = np.empty((B, S_, D), np.float32)
    for c in range(N_CORES):
        b, h = divmod(c, 2)
        outp = results[c]["outp"]  # [128, 2*SQ] fp16: [p, dot*SQ + s]
        v = outp.reshape(P, DT, SQ).astype(np.float32)
        out[b, h * SQ : (h + 1) * SQ] = v.transpose(2, 1, 0).reshape(SQ, D)
    return out.reshape(B, S_, H, W)


def kernel(x, Wq, bq, Wk, bk, Wv, bv, Wo, bo, _trace=False):
    from concourse.bass_utils import run_bass_kernel_spmd

    nc = _get_nc()
    in_maps = make_in_maps(x, Wq, bq, Wk, bk, Wv, bv, Wo, bo)
    res = run_bass_kernel_spmd(nc, in_maps, list(range(N_CORES)), trace=_trace)
    out = assemble_out(res.results, x.shape)
    if _trace:
        _CACHE["last_result"] = res
    return out



# revision 8
# speedup vs baseline: 1.3671x; 1.0127x over previous
"""Trainium2 Bass kernel for nn_MultiHeadAttention_59227599012491.

Reference computation (per batch b):
    xf = x[b].reshape(S, 256)
    q  = softplus(xf @ Wq.T + bq);  k = softplus(xf @ Wk.T + bk)
    v  = xf @ Wv.T + bv
    weight = q @ k.T            (no softmax!)
    result = weight @ v
    out    = result @ Wo.T + bo

Because there is no softmax, attention is associative:
    result = (q @ k.T) @ v = q @ (k.T @ v) = q @ G,   G: [256, 256]
    out    = q @ (G @ Wo.T) + bo = q @ M + bo
so the S x S score matrix never needs to be materialized. Per-core work
drops to a handful of [*, 256] x [256, 256] matmuls; the kernel is
jointly engine-bound (PE ~32 us, ACT ~29 us, DVE ~28 us busy/core).

Hard-won scheduling facts (all measured): the three DMA-issuing rings
(sync/scalar HWDGE, gpsimd SWDGE) share the same 16 physical DMA
engines, so the single sync-ring FIFO *is* the priority mechanism --
spreading input loads across rings only slows the critical pieces.
GPSIMD cannot touch PSUM; two PSUM accumulation groups must not share
a bank; an NRT AllReduce has ~17 us fixed rendezvous latency. fp32
PSUM evictions are uop-pinned to 1x on the DVE (one PSUM read port)
and ScalarE is 1 elem/cycle for all dtypes, so the elementwise floors
are hard. DMA-order rule: FIFO position in the single sync ring is
priority; order each engine's gate tensor so all gates of the first
dependent op complete SIMULTANEOUSLY (biasc right after x0 balances
the DVE's bias gate against its matmul gate -- either extreme costs
1-3 us on the DVE-paced loop). Descriptor count = DRAM row count per
tensor; the head floor is 512 descriptors (~5.5 us) before the first
matmul.

Sharding: B=4 batches x 2 query-halves -> 8 cores, no collectives.
(An NRT AllReduce of the tiny M matrix was measured at ~17 us fixed
rendezvous latency on this runtime -- more than the k/v/G dedup saves --
so each core recomputes k/v/G/M for its whole batch and only the
query/output rows are split across the pair.)

Layouts (PE computes out = lhsT.T @ rhs, contracting partition dim):
    xbT  [256, 4096]  x[b] transposed on host (queries first SQ cols)
    qT   [256, 2048]  lhsT = WqT tile, rhs = xbT     (softplus via ACT,
                      bias per-partition, fused into the Exp pass)
    kv   [4096, 512]  k and v fused: rhs = [WkT | WvT], one stationary
                      xbT tile per row tile serves both. +[bk|bv] via a
                      single DVE add; softplus on the k half in-place
                      (ACT Exp then Ln(1+t); batch-size ramp 2,4,8,8
                      then 4,2,2,2 -- small first so the ~98%-saturated
                      ACT engine starts ASAP, small last so the final
                      softplus -> GT -> M -> out chain stays short)
    GT   [256, 256]   GT[d,e] = sum_s v[s,d] k[s,e]: lhsT = v t, rhs = k t
    M    [256, 256]   M[e,do] = sum_d GT[d,e] WoT[d,do]: lhsT = GT, rhs = WoT
    outT [256, 2048]  transposed output: lhsT = M block (stationary,
                      reused across s), rhs = qT chunk; bo is then
                      per-partition (DVE tensor_scalar_add) and the fp16
                      dump has 2 KB descriptor runs -- half the output
                      bytes and a quarter of the descriptors vs the
                      natural-layout fp32 store; host un-transposes

The tile scheduler interleaves the qT/GT/out matmuls into the DVE-paced
kv loop's PE gaps, so the PE runs at ~91% occupancy over its window;
the engines are jointly near-saturated (PE ~32 us, ACT ~29 us, DVE
~28 us busy per core) and the phase structure below measures faster
than every explicitly-interleaved variant tried.

The activation-table pass is steered to `natural_log_exp_and_others`
(the only set holding Exp AND Ln) so the ACT engine loads its PWP table
once instead of reloading per activation (24 loads ~= 30us saved).
"""

import numpy as np

S = 4096
SQ = 2048  # query rows per core
D = 256
P = 128
IT = D // P  # 2 input-dim tiles
DT = D // P  # 2 d-model tiles
NS = S // P  # 32 sequence tiles
BLK = 512  # free-dim block for qT
N_CORES = 8

MM_DTYPE_NAME = "float16"

_CACHE = {}


def _patched_act_tables(orig_fn):
    def patched(arch):
        tabs = orig_fn(arch)
        return {
            name: (s if name == "natural_log_exp_and_others" else set())
            for name, s in tabs.items()
        }

    return patched


def _build_nc():
    import concourse.bacc as bacc
    import concourse.mybir as mybir
    import concourse.tile as tile

    FP = mybir.dt.float32
    FR = getattr(mybir.dt, MM_DTYPE_NAME)
    AF = mybir.ActivationFunctionType
    ADD = mybir.AluOpType.add

    nc = bacc.Bacc("TRN2", target_bir_lowering=False, debug=False, num_devices=1)

    xbT_d = nc.declare_dram_parameter("xbT", [D, S], FR, isOutput=False)
    # weights host-packed with it-blocks side by side: 2 KB descriptor rows
    wkvp_d = nc.declare_dram_parameter("wkvp", [P, 1024], FR, isOutput=False)
    wqop_d = nc.declare_dram_parameter("wqop", [P, 1024], FR, isOutput=False)
    # all biases in one [128, 516] fp32 tensor (cols 0:4 = bqT|boT,
    # 4:516 = host-replicated [bk|bv] row): one early DMA, 2 KB rows,
    # so the DVE and ACT queue gates both clear right after wkv
    bias_d = nc.declare_dram_parameter("biasc", [P, 4 + 2 * D], FP, isOutput=False)
    outp_d = nc.declare_dram_parameter("outp", [P, 2 * SQ], FR, isOutput=True)

    def mm(psum, lhsT, rhs, start, stop):
        nc.tensor.matmul(psum, lhsT, rhs, start=start, stop=stop)

    with tile.TileContext(nc) as tc:
        with (
            tc.tile_pool(name="w", bufs=1) as wpool,
            tc.tile_pool(name="big", bufs=1) as big,
            tc.tile_pool(name="tmp", bufs=4) as tpool,
            tc.tile_pool(name="psQ", bufs=3, space="PSUM") as psQ,
            tc.tile_pool(name="psKV", bufs=3, space="PSUM") as psKV,
            tc.tile_pool(name="psG", bufs=2, space="PSUM") as psG,
        ):
            # PE pstate priming: dependency-free matmuls on a scratch tile
            # ramp the clock out of its low pstate during the DMA head
            prime_sb = wpool.tile([P, 512], FR, tag="prime")
            nc.vector.memset(prime_sb[:, :], 0.0)
            psprime = psG.tile([P, D], FP, tag="psG")
            for _pi in range(6):
                nc.tensor.matmul(psprime[:, :], prime_sb[:, 0:P], prime_sb[:, 0:D], start=True, stop=True)
            wkv_sb = wpool.tile([P, 2 * 512], FR, tag="wkv")
            wqo_sb = wpool.tile([P, 2 * 512], FR, tag="wqo")
            xbT_sb = big.tile([P, IT, S], FR, tag="xbT")
            biasc = wpool.tile([P, 4 + 2 * D], FP, tag="biasc")
            bias_sb = biasc[:, 0:4]
            bc_bkv = biasc[:, 4 : 4 + 2 * D]
            b_bc = {"bkv": bc_bkv}
            nc.sync.dma_start(wkv_sb[:, :], wkvp_d.ap()[:, :])
            for it in range(IT):
                nc.sync.dma_start(
                    xbT_sb[:, it, 0:1024], xbT_d.ap()[it * P : (it + 1) * P, 0:1024]
                )
            # biasc directly after x0: the DVE's two gates (bias DMA and
            # the first tiles' matmuls) then complete simultaneously --
            # before x0 it over-serves the bias gate and the first matmul
            # slips ~1 us; after x1 the bias gate dominates by ~3 us
            nc.sync.dma_start(biasc[:, :], bias_d.ap()[:, :])
            # wqo before the second x chunk: the qT matmuls are the PE's
            # early gap-filler and their LDWEIGHTS gate on this tensor
            nc.sync.dma_start(wqo_sb[:, :], wqop_d.ap()[:, :])
            for it in range(IT):
                nc.sync.dma_start(
                    xbT_sb[:, it, 1024:2048], xbT_d.ap()[it * P : (it + 1) * P, 1024:2048]
                )
            for it in range(IT):
                nc.sync.dma_start(
                    xbT_sb[:, it, 2048:3072], xbT_d.ap()[it * P : (it + 1) * P, 2048:3072]
                )
            for it in range(IT):
                nc.sync.dma_start(
                    xbT_sb[:, it, 3072:4096], xbT_d.ap()[it * P : (it + 1) * P, 3072:4096]
                )

            kv_sb = big.tile([P, 2, NS, D], FR, tag="kv")
            qT_sb = big.tile([P, DT, SQ], FR, tag="qT")
            outT_sb = big.tile([P, DT, SQ], FR, tag="outT")
            GT_sb = wpool.tile([P, DT, D], FR, tag="GT")
            M_sb = wpool.tile([P, DT, D], FR, tag="M")

            for t in range(NS):
                ts = slice(t * P, (t + 1) * P)
                ps = psKV.tile([P, 2 * D], FP, tag="psKV")
                for it in range(IT):
                    mm(ps[:, :], xbT_sb[:, it, ts], wkv_sb[:, it * 512 : (it + 1) * 512], it == 0, it == IT - 1)
                nc.vector.tensor_tensor(
                    kv_sb[:, :, t, :], ps[:, :].rearrange("p (j d) -> p j d", j=2),
                    b_bc["bkv"][:, :].rearrange("p (j d) -> p j d", j=2), op=ADD,
                )
                # batch-size ramp: 2,4,8,8,8 then 2 at the end -- early
                # batches small so the saturated ACT engine starts ASAP,
                # final batches small so the softplus->GT->M->out chain
                # after the last kv tile stays short
                SPB = {1: 2, 5: 4, 13: 8, 21: 8, 25: 4, 27: 2, 29: 2, 31: 2}
                bsz = SPB.get(t, 0)
                if bsz:
                    tt = slice(t - bsz + 1, t + 1)
                    tmp = tpool.tile([P, bsz, D], FP, tag=f"tmpk{bsz}")
                    nc.scalar.activation(tmp[:, :, :], kv_sb[:, 0, tt, :], AF.Exp)
                    nc.scalar.activation(kv_sb[:, 0, tt, :], tmp[:, :, :], AF.Ln, bias=1.0)

            for dt in range(DT):
                for half in range(SQ // (2 * BLK)):
                    tmp = tpool.tile([P, 2, BLK], FP, tag="tmpq")
                    for c in range(2):
                        blk = 2 * half + c
                        ss = slice(blk * BLK, (blk + 1) * BLK)
                        ps = psQ.tile([P, BLK], FP, tag="psQ")
                        for it in range(IT):
                            mm(ps[:, :], wqo_sb[:, it * D + dt * P : it * D + (dt + 1) * P], xbT_sb[:, it, ss], it == 0, it == IT - 1)
                        nc.scalar.activation(
                            tmp[:, c, :], ps[:, :], AF.Exp, bias=bias_sb[:, dt : dt + 1]
                        )
                    nc.scalar.activation(
                        qT_sb[:, dt, 2 * half * BLK : 2 * (half + 1) * BLK],
                        tmp[:, :, :].rearrange("p a b -> p (a b)"),
                        AF.Ln,
                        bias=1.0,
                    )

            for dt in range(DT):
                vs = slice(dt * P, (dt + 1) * P)
                ps = psG.tile([P, D], FP, tag="psG")
                for t in range(NS):
                    mm(ps[:, :], kv_sb[:, 1, t, vs], kv_sb[:, 0, t, :], t == 0, t == NS - 1)
                nc.vector.tensor_copy(GT_sb[:, dt, :], ps[:, :])

            for et in range(DT):
                es = slice(et * P, (et + 1) * P)
                ps = psG.tile([P, D], FP, tag="psG")
                for dt in range(DT):
                    mm(ps[:, :], GT_sb[:, dt, es], wqo_sb[:, 512 + dt * D : 512 + (dt + 1) * D], dt == 0, dt == DT - 1)
                nc.vector.tensor_copy(M_sb[:, et, :], ps[:, :])

            # outT[do, s] = M^T q^T + bo: lhsT = M block (stationary,
            # reused across s), per-partition bo bias on the DVE, fp16
            # transposed dump with 2 KB descriptor runs
            for dot in range(DT):
                for blk in range(SQ // BLK):
                    ss = slice(blk * BLK, (blk + 1) * BLK)
                    ps = psQ.tile([P, BLK], FP, tag="psQ")
                    for et in range(DT):
                        mm(
                            ps[:, :],
                            M_sb[:, et, dot * P : (dot + 1) * P],
                            qT_sb[:, et, ss],
                            et == 0,
                            et == DT - 1,
                        )
                    if dot == DT - 1 and blk == SQ // BLK - 1:
                        # very last chunk: split the eviction across the idle
                        # ACT and DVE so the serial tail halves
                        nc.scalar.activation(
                            outT_sb[:, dot, blk * BLK : blk * BLK + 256],
                            ps[:, 0:256],
                            AF.Identity,
                            bias=bias_sb[:, 2 + dot : 3 + dot],
                        )
                        nc.vector.tensor_scalar_add(
                            outT_sb[:, dot, blk * BLK + 256 : (blk + 1) * BLK],
                            ps[:, 256:512],
                            bias_sb[:, 2 + dot : 3 + dot],
                        )
                    elif dot == DT - 1:
                        # ACT is drained by now; Identity(in + bo) keeps the
                        # final eviction off the backlogged DVE queue
                        nc.scalar.activation(
                            outT_sb[:, dot, ss],
                            ps[:, :],
                            AF.Identity,
                            bias=bias_sb[:, 2 + dot : 3 + dot],
                        )
                    else:
                        nc.vector.tensor_scalar_add(
                            outT_sb[:, dot, ss], ps[:, :], bias_sb[:, 2 + dot : 3 + dot]
                        )
                    if dot == DT - 1 and blk >= SQ // BLK - 2:
                        # ship the last two chunks individually so only one
                        # chunk's descriptors remain after the final eviction
                        off = dot * SQ + blk * BLK
                        src_ap = outT_sb[:, dot, blk * BLK : (blk + 1) * BLK]
                        if blk == SQ // BLK - 1:
                            nc.sync.dma_start(
                                outp_d.ap()[0:64, off : off + BLK], src_ap[0:64, :]
                            )
                            nc.scalar.dma_start(
                                outp_d.ap()[64:P, off : off + BLK], src_ap[64:P, :]
                            )
                        else:
                            nc.sync.dma_start(outp_d.ap()[:, off : off + BLK], src_ap)
                    elif blk % 2 == 1:
                        off = dot * SQ + (blk - 1) * BLK
                        src_ap = outT_sb[:, dot, (blk - 1) * BLK : (blk + 1) * BLK]
                        nc.sync.dma_start(
                            outp_d.ap()[:, off : off + 2 * BLK], src_ap
                        )

    import concourse.hw_specs as hw_specs

    orig = bacc.get_activation_tables
    bacc.get_activation_tables = _patched_act_tables(hw_specs.get_activation_tables)
    try:
        nc.compile()
    finally:
        bacc.get_activation_tables = orig
    return nc


def _get_nc():
    nc = _CACHE.get("nc")
    if nc is None:
        nc = _build_nc()
        _CACHE["nc"] = nc
    return nc


def make_in_maps(x, Wq, bq, Wk, bk, Wv, bv, Wo, bo):
    B = x.shape[0]
    mmnp = np.float16
    xf = np.asarray(x, dtype=np.float32).reshape(B, S, D)
    xfT = np.ascontiguousarray(xf.transpose(0, 2, 1).astype(mmnp))
    wkv2 = np.hstack([np.asarray(Wk, mmnp).T, np.asarray(Wv, mmnp).T])  # [256, 512]
    wkvp = np.ascontiguousarray(
        wkv2.reshape(2, P, 512).transpose(1, 0, 2).reshape(P, 1024)
    )
    wq2 = np.asarray(Wq, mmnp).T
    wo2 = np.asarray(Wo, mmnp).T
    wqop = np.ascontiguousarray(
        np.hstack([wq2[0:P], wq2[P:D], wo2[0:P], wo2[P:D]])
    )  # [128, (it0 wq|it1 wq|dt0 wo|dt1 wo)]
    bkvrow = np.concatenate([np.asarray(bk, np.float32), np.asarray(bv, np.float32)])
    biasc = np.ascontiguousarray(
        np.hstack(
            [
                np.stack(
                    [
                        np.asarray(bq, np.float32)[0:P],
                        np.asarray(bq, np.float32)[P:D],
                        np.asarray(bo, np.float32)[0:P],
                        np.asarray(bo, np.float32)[P:D],
                    ],
                    axis=1,
                ),
                np.tile(bkvrow, (P, 1)),
            ]
        )
    )
    shared = {
        "wkvp": wkvp,
        "wqop": wqop,
        "biasc": biasc,
    }
    in_maps = []
    for c in range(N_CORES):
        b, h = divmod(c, 2)
        xT = xfT[b]
        if h == 1:
            xT = np.concatenate([xT[:, SQ:], xT[:, :SQ]], axis=1)
        in_maps.append({"xbT": np.ascontiguousarray(xT), **shared})
    return in_maps


def assemble_out(results, x_shape):
    B, S_, H, W = x_shape
    out = np.empty((B, S_, D), np.float32)
    for c in range(N_CORES):
        b, h = divmod(c, 2)
        outp = results[c]["outp"]  # [128, 2*SQ] fp16: [p, dot*SQ + s]
        v = outp.reshape(P, DT, SQ).astype(np.float32)
        out[b, h * SQ : (h + 1) * SQ] = v.transpose(2, 1, 0).reshape(SQ, D)
    return out.reshape(B, S_, H, W)


def kernel(x, Wq, bq, Wk, bk, Wv, bv, Wo, bo, _trace=False):
    from concourse.bass_utils import run_bass_kernel_spmd

    nc = _get_nc()
    in_maps = make_in_maps(x, Wq, bq, Wk, bk, Wv, bv, Wo, bo)
    res = run_bass_kernel_spmd(nc, in_maps, list(range(N_CORES)), trace=_trace)
    out = assemble_out(res.results, x.shape)
    if _trace:
        _CACHE["last_result"] = res
    return out

